# revision 9
# baseline (speedup 1.0000x reference)
"""GQA attention kernel for Trainium2, sharded over 8 NeuronCores.

Problem: B=4, S=2048, E=2048, 16 q heads / 4 kv heads, head_dim=128,
full (non-causal) softmax attention with additive sincos pos emb on Q/K.

Sharding: core c = 2*b + j  (b in 0..3 batch, j in 0..1 head-half).
Each core computes 8 query heads (= 2 kv-head groups) for one batch
element: Q/K/V projections restricted to its head slice, then attention.
Outputs are disjoint slices of the full [4, 2048, 2048] output.

Kernel layout trick: all projections are computed transposed
(QT = Wq^T @ x^T etc. with the weight as the stationary operand), and
attention scores are computed transposed (scoresT[key, query] with KT
stationary) so that after the exp, probsT is already in the stationary
(lhsT) layout needed for the probs @ V matmul -- no on-chip transposes.
Softmax is unnormalized (exp without max subtraction; scores are O(1)
for this distribution) with the row-sum obtained from an appended
ones-column in V; normalization is a per-partition reciprocal multiply
on the final [q,d] tile.
"""

import sys
from contextlib import ExitStack

sys.path.insert(0, "/opt/trn_rl_repo")

import numpy as np
import ml_dtypes

import concourse.bass as bass
import concourse.mybir as mybir
import concourse.tile as tile
from concourse import bacc
from concourse.bass_utils import run_bass_kernel_spmd

# problem constants (hardcoded per contract)
B, S, E = 4, 2048, 2048
H, HKV, D = 16, 4, 128
REP = H // HKV  # 4
N_CORES = 8
HPC = 8          # q heads per core
KVPC = 2         # kv heads per core
MQ = HPC * D     # 1024 q-proj output dim per core
MKV = KVPC * D   # 256 kv-proj output dim per core
KT_TILES = E // 128   # 16 contraction tiles
ST_TILES = S // 128   # 16 key tiles
NCB = S // 512        # 4 free-dim chunks of 512
SCALE = 1.0 / float(np.sqrt(D))

BF16 = mybir.dt.bfloat16
FP32 = mybir.dt.float32
NP_BF16 = ml_dtypes.bfloat16

# test-harness hooks: test.py sets _RUN_KWARGS and reads _LAST_* for
# profiling; the grading path leaves these untouched.
_RUN_KWARGS: dict = {}
_LAST_RES = None
_LAST_NC = None
_LAST_INMAPS = None


def _build_program():
    nc = bacc.Bacc("TRN2", target_bir_lowering=False, debug=False,
                   num_devices=N_CORES)

    xT_d = nc.dram_tensor("xT", [E, S], BF16, kind="ExternalInput").ap()
    wq_d = nc.dram_tensor("wq", [E, MQ], BF16, kind="ExternalInput").ap()
    wk_d = nc.dram_tensor("wk", [E, MKV], BF16, kind="ExternalInput").ap()
    wv_d = nc.dram_tensor("wv", [E, MKV], BF16, kind="ExternalInput").ap()
    posqT_d = nc.dram_tensor("posqT", [MQ, S], BF16, kind="ExternalInput").ap()
    poskT_d = nc.dram_tensor("poskT", [MKV, S], BF16, kind="ExternalInput").ap()
    bvb_d = nc.dram_tensor("bvb", [128, MKV], FP32, kind="ExternalInput").ap()
    out_d = nc.dram_tensor("out", [S, MQ], FP32, kind="ExternalOutput").ap()

    with tile.TileContext(nc) as tc, ExitStack() as ctx:
        xT_pool = ctx.enter_context(tc.tile_pool(name="xT", bufs=KT_TILES))
        wq_pool = ctx.enter_context(tc.tile_pool(name="wq", bufs=KT_TILES))
        wkv_pool = ctx.enter_context(tc.tile_pool(name="wkv", bufs=2 * KT_TILES))
        posq_pool = ctx.enter_context(tc.tile_pool(name="posq", bufs=2))
        posk_pool = ctx.enter_context(tc.tile_pool(name="posk", bufs=2))
        kt_pool = ctx.enter_context(tc.tile_pool(name="KT", bufs=KVPC))
        qt_pool = ctx.enter_context(tc.tile_pool(name="QT", bufs=HPC))
        v_pool = ctx.enter_context(tc.tile_pool(name="V", bufs=ST_TILES))
        bv_pool = ctx.enter_context(tc.tile_pool(name="bv", bufs=1))
        probs_pool = ctx.enter_context(tc.tile_pool(name="probs", bufs=6))
        outt_pool = ctx.enter_context(tc.tile_pool(name="outt", bufs=4))
        rec_pool = ctx.enter_context(tc.tile_pool(name="rec", bufs=4))
        ps_pool = ctx.enter_context(
            tc.tile_pool(name="psmm", bufs=3, space="PSUM"))
        pso_pool = ctx.enter_context(
            tc.tile_pool(name="psout", bufs=4, space="PSUM"))

        # ---- resident loads ----
        xT = []
        for k in range(KT_TILES):
            t = xT_pool.tile([128, S], BF16, tag="xT", name=f"xT{k}")
            nc.sync.dma_start(out=t[:], in_=xT_d[k * 128:(k + 1) * 128, :])
            xT.append(t)
        wq_t = []
        for k in range(KT_TILES):
            t = wq_pool.tile([128, MQ], BF16, tag="wq", name=f"wq{k}")
            nc.sync.dma_start(out=t[:], in_=wq_d[k * 128:(k + 1) * 128, :])
            wq_t.append(t)
        wk_t, wv_t = [], []
        for k in range(KT_TILES):
            t = wkv_pool.tile([128, MKV], BF16, tag="wkv", name=f"wk{k}")
            nc.sync.dma_start(out=t[:], in_=wk_d[k * 128:(k + 1) * 128, :])
            wk_t.append(t)
            t = wkv_pool.tile([128, MKV], BF16, tag="wkv", name=f"wv{k}")
            nc.sync.dma_start(out=t[:], in_=wv_d[k * 128:(k + 1) * 128, :])
            wv_t.append(t)
        bv_sb = bv_pool.tile([128, MKV], FP32, tag="bv")
        nc.sync.dma_start(out=bv_sb[:], in_=bvb_d[:])

        KT_sb = [kt_pool.tile([128, S], BF16, tag="KT", name=f"KT{i}") for i in range(KVPC)]
        QT_sb = [qt_pool.tile([128, S], BF16, tag="QT", name=f"QT{i}") for i in range(HPC)]
        # V tiles: per s-tile, layout [head0 d0..127 | ones | head1 d0..127 | ones]
        VW = D + 1
        V_sb = [v_pool.tile([128, KVPC * VW], BF16, tag="V", name=f"V{i}")
                for i in range(ST_TILES)]

        # ---- K projection: KT[m][128, S] = (Wk^T @ xT)[m] + poskT ----
        for m in range(KVPC):
            pk = posk_pool.tile([128, S], BF16, tag="posk")
            nc.sync.dma_start(out=pk[:], in_=poskT_d[m * 128:(m + 1) * 128, :])
            for nb in range(NCB):
                ps = ps_pool.tile([128, 512], FP32, tag="psmm")
                for k in range(KT_TILES):
                    nc.tensor.matmul(
                        ps[:], wk_t[k][:, m * 128:(m + 1) * 128],
                        xT[k][:, nb * 512:(nb + 1) * 512],
                        start=(k == 0), stop=(k == KT_TILES - 1))
                nc.vector.tensor_add(
                    KT_sb[m][:, nb * 512:(nb + 1) * 512], ps[:],
                    pk[:, nb * 512:(nb + 1) * 512])

        # ---- V projection: V[st][128, 256] = (x @ Wv)[st] + bv ----
        for st in range(ST_TILES):
            ps = ps_pool.tile([128, MKV], FP32, tag="psmm")
            for k in range(KT_TILES):
                nc.tensor.matmul(
                    ps[:], xT[k][:, st * 128:(st + 1) * 128], wv_t[k][:],
                    start=(k == 0), stop=(k == KT_TILES - 1))
            for g in range(KVPC):
                nc.vector.tensor_add(
                    V_sb[st][:, g * VW:g * VW + D],
                    ps[:, g * D:(g + 1) * D], bv_sb[:, g * D:(g + 1) * D])
                nc.vector.memset(V_sb[st][:, g * VW + D:(g + 1) * VW], 1.0)

        # ---- Q projection: QT[h][128, S] = (Wq^T @ xT)[h] + posqT ----
        for m in range(HPC):
            pq = posq_pool.tile([128, S], BF16, tag="posq")
            nc.sync.dma_start(out=pq[:], in_=posqT_d[m * 128:(m + 1) * 128, :])
            for nb in range(NCB):
                ps = ps_pool.tile([128, 512], FP32, tag="psmm")
                for k in range(KT_TILES):
                    nc.tensor.matmul(
                        ps[:], wq_t[k][:, m * 128:(m + 1) * 128],
                        xT[k][:, nb * 512:(nb + 1) * 512],
                        start=(k == 0), stop=(k == KT_TILES - 1))
                nc.vector.tensor_add(
                    QT_sb[m][:, nb * 512:(nb + 1) * 512], ps[:],
                    pq[:, nb * 512:(nb + 1) * 512])

        # ---- attention, per (q head, 512-wide q chunk) ----
        for h in range(HPC):
            g = h // REP  # local kv head
            for qc in range(NCB):
                qs = qc * 512
                # scoresT[st][128 keys, 512 queries] -> exp -> probsT (bf16)
                pts = []
                for st in range(ST_TILES):
                    ps = ps_pool.tile([128, 512], FP32, tag="psmm")
                    nc.tensor.matmul(
                        ps[:], KT_sb[g][:, st * 128:(st + 1) * 128],
                        QT_sb[h][:, qs:qs + 512], start=True, stop=True)
                    pt = probs_pool.tile([128, 512], BF16, tag="probs")
                    nc.scalar.activation(
                        pt[:], ps[:], mybir.ActivationFunctionType.Exp,
                        bias=0.0, scale=SCALE)
                    pts.append(pt)
                # out_aug[q 128, 129] = probsT.T @ [V | 1]; st-outer so the
                # PE consumes each probsT tile right after its exp finishes
                psos = [pso_pool.tile([128, VW], FP32, tag="psout",
                                       name=f"pso{h}_{qc}_{i}")
                        for i in range(4)]
                for st in range(ST_TILES):
                    for qt in range(4):
                        nc.tensor.matmul(
                            psos[qt][:],
                            pts[st][:, qt * 128:(qt + 1) * 128],
                            V_sb[st][:, g * VW:(g + 1) * VW],
                            start=(st == 0), stop=(st == ST_TILES - 1))
                for qt in range(4):
                    rec = rec_pool.tile([128, 1], FP32, tag="rec")
                    nc.vector.reciprocal(rec[:], psos[qt][:, D:D + 1])
                    ot = outt_pool.tile([128, D], FP32, tag="outt")
                    nc.vector.tensor_scalar_mul(ot[:], psos[qt][:, 0:D], rec[:])
                    q0 = qs + qt * 128
                    nc.sync.dma_start(
                        out=out_d[q0:q0 + 128, h * 128:(h + 1) * 128],
                        in_=ot[:])
    nc.compile()
    return nc


def kernel(x, wq_w, wq_b, wk_w, wk_b, wv_w, wv_b, pos_emb):
    x = np.asarray(x, dtype=np.float32)
    wq_w = np.asarray(wq_w, dtype=np.float32)
    wq_b = np.asarray(wq_b, dtype=np.float32)
    wk_w = np.asarray(wk_w, dtype=np.float32)
    wk_b = np.asarray(wk_b, dtype=np.float32)
    wv_w = np.asarray(wv_w, dtype=np.float32)
    wv_b = np.asarray(wv_b, dtype=np.float32)
    pos_emb = np.asarray(pos_emb, dtype=np.float32)

    nc = _build_program()

    in_maps = []
    for c in range(N_CORES):
        b, j = c // 2, c % 2
        qsl = slice(MQ * j, MQ * (j + 1))      # q-head columns
        ksl = slice(MKV * j, MKV * (j + 1))    # kv-head columns
        posq = pos_emb[:, qsl] + wq_b[qsl][None, :]       # [S, 1024]
        posk = pos_emb[:, ksl] + wk_b[ksl][None, :]       # [S, 256] (first 512 cols of pos cover kv)
        bvb = np.broadcast_to(wv_b[ksl][None, :], (128, MKV)).copy()
        in_maps.append({
            "xT": np.ascontiguousarray(x[b].T).astype(NP_BF16),
            "wq": wq_w[:, qsl].astype(NP_BF16),
            "wk": wk_w[:, ksl].astype(NP_BF16),
            "wv": wv_w[:, ksl].astype(NP_BF16),
            "posqT": np.ascontiguousarray(posq.T).astype(NP_BF16),
            "poskT": np.ascontiguousarray(posk.T).astype(NP_BF16),
            "bvb": bvb.astype(np.float32),
        })

    globals()["_LAST_NC"] = nc
    globals()["_LAST_INMAPS"] = in_maps
    res = run_bass_kernel_spmd(nc, in_maps, list(range(N_CORES)), **_RUN_KWARGS)
    globals()["_LAST_RES"] = res

    out = np.empty((B, S, E), dtype=np.float32)
    for c in range(N_CORES):
        b, j = c // 2, c % 2
        out[b, :, MQ * j:MQ * (j + 1)] = res.results[c]["out"]
    return out


# revision 10
# speedup vs baseline: 1.0516x; 1.0516x over previous
"""GQA attention kernel for Trainium2, sharded over 8 NeuronCores.

Problem: B=4, S=2048, E=2048, 16 q heads / 4 kv heads, head_dim=128,
full (non-causal) softmax attention with additive sincos pos emb on Q/K.

Sharding: core c = 2*b + j  (b in 0..3 batch, j in 0..1 head-half).
Each core computes 8 query heads (= 2 kv-head groups) for one batch
element. Outputs are disjoint slices of the full [4, 2048, 2048] output.

Layout: projections are computed transposed (QT = Wq^T @ x^T with the
weight stationary) and scores transposed (scoresT[key, query], KT
stationary), so post-exp probsT is already in the stationary (lhsT)
layout for the probs @ V matmul -- no on-chip transposes. Softmax skips
the max-subtraction (scores are O(1) here); the row-sum falls out of an
appended ones-column in V and normalization is a per-partition
reciprocal multiply at the end.

Schedule: the exp over the 8*2048*2048 score matrix saturates ScalarE
(~300us) while the projections saturate TensorE, so Q-projection
matmuls for head h+1 are interleaved into the attention matmul stream
of head h -- TensorE then hides all ScalarE work instead of the two
phases serializing. DMAs are emitted in consumption order (wk, first
xT column-chunk, ...) so the first matmul starts ~10us in.
"""

import sys
from contextlib import ExitStack

sys.path.insert(0, "/opt/trn_rl_repo")

import numpy as np
import ml_dtypes

import concourse.bass as bass
import concourse.mybir as mybir
import concourse.tile as tile
from concourse import bacc
from concourse.bass_utils import run_bass_kernel_spmd

# problem constants (hardcoded per contract)
B, S, E = 4, 2048, 2048
H, HKV, D = 16, 4, 128
REP = H // HKV  # 4
N_CORES = 8
HPC = 8          # q heads per core
KVPC = 2         # kv heads per core
MQ = HPC * D     # 1024 q-proj output dim per core
MKV = KVPC * D   # 256 kv-proj output dim per core
KT_TILES = E // 128   # 16 contraction tiles
ST_TILES = S // 128   # 16 key tiles
NCB = S // 512        # 4 free-dim chunks of 512
SCALE = 1.0 / float(np.sqrt(D))
VW = D + 1            # V block width incl. ones column

BF16 = mybir.dt.bfloat16
FP32 = mybir.dt.float32
NP_BF16 = ml_dtypes.bfloat16

# test-harness hooks: test.py sets _RUN_KWARGS and reads _LAST_* for
# profiling; the grading path leaves these untouched.
_RUN_KWARGS: dict = {}
_LAST_RES = None
_LAST_NC = None
_LAST_INMAPS = None


def _build_program():
    nc = bacc.Bacc("TRN2", target_bir_lowering=False, debug=False,
                   num_devices=N_CORES)

    xT_d = nc.dram_tensor("xT", [E, S], BF16, kind="ExternalInput").ap()
    wq_d = nc.dram_tensor("wq", [E, MQ], BF16, kind="ExternalInput").ap()
    wk_d = nc.dram_tensor("wk", [E, MKV], BF16, kind="ExternalInput").ap()
    wv_d = nc.dram_tensor("wv", [E, MKV], BF16, kind="ExternalInput").ap()
    posqT_d = nc.dram_tensor("posqT", [MQ, S], BF16, kind="ExternalInput").ap()
    poskT_d = nc.dram_tensor("poskT", [MKV, S], BF16, kind="ExternalInput").ap()
    bvb_d = nc.dram_tensor("bvb", [128, MKV], FP32, kind="ExternalInput").ap()
    out_d = nc.dram_tensor("out", [S, MQ], FP32, kind="ExternalOutput").ap()

    with tile.TileContext(nc) as tc, ExitStack() as ctx:
        xT_pool = ctx.enter_context(tc.tile_pool(name="xT", bufs=KT_TILES * NCB))
        wq_pool = ctx.enter_context(tc.tile_pool(name="wq", bufs=KT_TILES))
        wkv_pool = ctx.enter_context(tc.tile_pool(name="wkv", bufs=2 * KT_TILES))
        posq_pool = ctx.enter_context(tc.tile_pool(name="posq", bufs=2))
        posk_pool = ctx.enter_context(tc.tile_pool(name="posk", bufs=2))
        kt_pool = ctx.enter_context(tc.tile_pool(name="KT", bufs=KVPC))
        qt_pool = ctx.enter_context(tc.tile_pool(name="QT", bufs=HPC))
        v_pool = ctx.enter_context(tc.tile_pool(name="V", bufs=ST_TILES))
        bv_pool = ctx.enter_context(tc.tile_pool(name="bv", bufs=1))
        probs_pool = ctx.enter_context(tc.tile_pool(name="probs", bufs=6))
        outt_pool = ctx.enter_context(tc.tile_pool(name="outt", bufs=4))
        rec_pool = ctx.enter_context(tc.tile_pool(name="rec", bufs=4))
        # PSUM: 8 banks total. qp (proj accum) 2 + sc (scores) 2 + psout 4.
        qp_pool = ctx.enter_context(
            tc.tile_pool(name="qp", bufs=2, space="PSUM"))
        sc_pool = ctx.enter_context(
            tc.tile_pool(name="sc", bufs=2, space="PSUM"))
        pso_pool = ctx.enter_context(
            tc.tile_pool(name="psout", bufs=4, space="PSUM"))

        # ---- resident loads, in consumption order ----
        # K proj consumes wk + xT column-chunk 0 first.
        wk_t = []
        for k in range(KT_TILES):
            t = wkv_pool.tile([128, MKV], BF16, tag="wkv", name=f"wk{k}")
            nc.sync.dma_start(out=t[:], in_=wk_d[k * 128:(k + 1) * 128, :])
            wk_t.append(t)
        poskt = []
        for m in range(KVPC):
            t = posk_pool.tile([128, S], BF16, tag="posk", name=f"posk{m}")
            nc.sync.dma_start(out=t[:], in_=poskT_d[m * 128:(m + 1) * 128, :])
            poskt.append(t)
        # xT as [128, 512] chunks: xT[k][nb]
        xT = [[None] * NCB for _ in range(KT_TILES)]
        for nb in range(NCB):
            for k in range(KT_TILES):
                t = xT_pool.tile([128, 512], BF16, tag="xT",
                                 name=f"xT{k}_{nb}")
                nc.sync.dma_start(
                    out=t[:],
                    in_=xT_d[k * 128:(k + 1) * 128, nb * 512:(nb + 1) * 512])
                xT[k][nb] = t
        wv_t = []
        for k in range(KT_TILES):
            t = wkv_pool.tile([128, MKV], BF16, tag="wkv", name=f"wv{k}")
            nc.sync.dma_start(out=t[:], in_=wv_d[k * 128:(k + 1) * 128, :])
            wv_t.append(t)
        bv_sb = bv_pool.tile([128, MKV], FP32, tag="bv")
        nc.sync.dma_start(out=bv_sb[:], in_=bvb_d[:])
        wq_t = []
        for k in range(KT_TILES):
            t = wq_pool.tile([128, MQ], BF16, tag="wq", name=f"wq{k}")
            nc.sync.dma_start(out=t[:], in_=wq_d[k * 128:(k + 1) * 128, :])
            wq_t.append(t)

        KT_sb = [kt_pool.tile([128, S], BF16, tag="KT", name=f"KT{i}")
                 for i in range(KVPC)]
        QT_sb = [qt_pool.tile([128, S], BF16, tag="QT", name=f"QT{i}")
                 for i in range(HPC)]
        # V tiles: [head0 d0..127 | ones | head1 d0..127 | ones]
        V_sb = [v_pool.tile([128, KVPC * VW], BF16, tag="V", name=f"V{i}")
                for i in range(ST_TILES)]

        # ---- K projection ----
        for m in range(KVPC):
            for nb in range(NCB):
                ps = qp_pool.tile([128, 512], FP32, tag="qp")
                for k in range(KT_TILES):
                    nc.tensor.matmul(
                        ps[:], wk_t[k][:, m * 128:(m + 1) * 128], xT[k][nb][:],
                        start=(k == 0), stop=(k == KT_TILES - 1))
                nc.vector.tensor_add(
                    KT_sb[m][:, nb * 512:(nb + 1) * 512], ps[:],
                    poskt[m][:, nb * 512:(nb + 1) * 512])

        # ---- V projection ----
        for st in range(ST_TILES):
            ps = qp_pool.tile([128, MKV], FP32, tag="qp")
            for k in range(KT_TILES):
                nc.tensor.matmul(
                    ps[:], xT[k][st // 4][:, (st % 4) * 128:(st % 4 + 1) * 128],
                    wv_t[k][:], start=(k == 0), stop=(k == KT_TILES - 1))
            for g in range(KVPC):
                nc.vector.tensor_add(
                    V_sb[st][:, g * VW:g * VW + D],
                    ps[:, g * D:(g + 1) * D], bv_sb[:, g * D:(g + 1) * D])
                nc.vector.memset(V_sb[st][:, g * VW + D:(g + 1) * VW], 1.0)

        # ---- Q projection, emitted as resumable steps ----
        def qproj_steps(m):
            """Yield once per matmul for QT[m]; epilogues ride along."""
            pq = posq_pool.tile([128, S], BF16, tag="posq", name=f"posq{m}")
            nc.sync.dma_start(out=pq[:], in_=posqT_d[m * 128:(m + 1) * 128, :])
            for nb in range(NCB):
                ps = qp_pool.tile([128, 512], FP32, tag="qp",
                                  name=f"qps{m}_{nb}")
                for k in range(KT_TILES):
                    nc.tensor.matmul(
                        ps[:], wq_t[k][:, m * 128:(m + 1) * 128], xT[k][nb][:],
                        start=(k == 0), stop=(k == KT_TILES - 1))
                    yield
                nc.vector.tensor_add(
                    QT_sb[m][:, nb * 512:(nb + 1) * 512], ps[:],
                    pq[:, nb * 512:(nb + 1) * 512])

        def chain(gens):
            for g in gens:
                yield from g

        # head 0 projected up front; m=1..7 interleaved into attention
        for _ in qproj_steps(0):
            pass
        qgen = chain([qproj_steps(m) for m in range(1, HPC)])

        def qstep(n=1):
            for _ in range(n):
                if next(qgen, None) is None:
                    return

        # ---- attention ----
        for h in range(HPC):
            g = h // REP
            for qc in range(NCB):
                qs = qc * 512
                pts = []
                for st in range(ST_TILES):
                    ps = sc_pool.tile([128, 512], FP32, tag="sc")
                    nc.tensor.matmul(
                        ps[:], KT_sb[g][:, st * 128:(st + 1) * 128],
                        QT_sb[h][:, qs:qs + 512], start=True, stop=True)
                    pt = probs_pool.tile([128, 512], BF16, tag="probs")
                    nc.scalar.activation(
                        pt[:], ps[:], mybir.ActivationFunctionType.Exp,
                        bias=0.0, scale=SCALE)
                    pts.append(pt)
                    qstep(1)
                psos = [pso_pool.tile([128, VW], FP32, tag="psout",
                                      name=f"pso{h}_{qc}_{i}")
                        for i in range(4)]
                for st in range(ST_TILES):
                    for qt in range(4):
                        nc.tensor.matmul(
                            psos[qt][:],
                            pts[st][:, qt * 128:(qt + 1) * 128],
                            V_sb[st][:, g * VW:(g + 1) * VW],
                            start=(st == 0), stop=(st == ST_TILES - 1))
                    qstep(1)
                for qt in range(4):
                    rec = rec_pool.tile([128, 1], FP32, tag="rec")
                    nc.vector.reciprocal(rec[:], psos[qt][:, D:D + 1])
                    ot = outt_pool.tile([128, D], FP32, tag="outt")
                    nc.vector.tensor_scalar_mul(ot[:], psos[qt][:, 0:D], rec[:])
                    q0 = qs + qt * 128
                    nc.sync.dma_start(
                        out=out_d[q0:q0 + 128, h * 128:(h + 1) * 128],
                        in_=ot[:])
        # drain any remaining q-proj steps (h=7 has no successor)
        qstep(10 ** 6)
    nc.compile()
    return nc


def kernel(x, wq_w, wq_b, wk_w, wk_b, wv_w, wv_b, pos_emb):
    x = np.asarray(x, dtype=np.float32)
    wq_w = np.asarray(wq_w, dtype=np.float32)
    wq_b = np.asarray(wq_b, dtype=np.float32)
    wk_w = np.asarray(wk_w, dtype=np.float32)
    wk_b = np.asarray(wk_b, dtype=np.float32)
    wv_w = np.asarray(wv_w, dtype=np.float32)
    wv_b = np.asarray(wv_b, dtype=np.float32)
    pos_emb = np.asarray(pos_emb, dtype=np.float32)

    nc = _build_program()

    in_maps = []
    for c in range(N_CORES):
        b, j = c // 2, c % 2
        qsl = slice(MQ * j, MQ * (j + 1))      # q-head columns
        ksl = slice(MKV * j, MKV * (j + 1))    # kv-head columns
        posq = pos_emb[:, qsl] + wq_b[qsl][None, :]       # [S, 1024]
        posk = pos_emb[:, ksl] + wk_b[ksl][None, :]       # [S, 256]
        bvb = np.broadcast_to(wv_b[ksl][None, :], (128, MKV)).copy()
        in_maps.append({
            "xT": np.ascontiguousarray(x[b].T).astype(NP_BF16),
            "wq": wq_w[:, qsl].astype(NP_BF16),
            "wk": wk_w[:, ksl].astype(NP_BF16),
            "wv": wv_w[:, ksl].astype(NP_BF16),
            "posqT": np.ascontiguousarray(posq.T).astype(NP_BF16),
            "poskT": np.ascontiguousarray(posk.T).astype(NP_BF16),
            "bvb": bvb.astype(np.float32),
        })

    globals()["_LAST_NC"] = nc
    globals()["_LAST_INMAPS"] = in_maps
    res = run_bass_kernel_spmd(nc, in_maps, list(range(N_CORES)), **_RUN_KWARGS)
    globals()["_LAST_RES"] = res

    out = np.empty((B, S, E), dtype=np.float32)
    for c in range(N_CORES):
        b, j = c // 2, c % 2
        out[b, :, MQ * j:MQ * (j + 1)] = res.results[c]["out"]
    return out
